# revision 4
# baseline (speedup 1.0000x reference)
"""Block-sparse MoE (SwiGLU, top-k of 8 experts) on 8 Trainium2 NeuronCores.

Sharding: FFN-dim (tensor-parallel within every expert).
  - Routing (gate matmul + softmax + top-k, ~0.07% of total FLOPs) runs on
    the host; tokens are grouped per expert (exact counts, tiny padding).
  - Every core processes ALL expert token batches, but only a 512-row slice
    of each expert's w1/w3 (and the matching w2 rows). Work is therefore
    identical across cores regardless of the routing distribution -- the
    per-expert load imbalance of expert-parallel dispatch disappears
    (~9% of tensor time for this routing).
  - Core partial outputs are summed on the host, then scatter-added into
    the full [T, H] output.

Device kernel per core (slots s=0..7 are experts sorted by descending
token count C_s; FL=512 ffn rows per expert per core):
  phase 1: interT[f, c] = silu(w1s @ xT) * (w3s @ xT)   (PSUM-accum over H)
  phase 2: yT[h, c]     = (w2s.T @ interT) * wgt[c]     (PSUM-accum over FL)
Matmuls default to fp16 (measured end-to-end rel err ~5e-4). MOE_DT can
select bf16 / f32 / f32r. All DRAM->SBUF transfers are host-pre-tiled so
every DMA is contiguous.
"""

import math
import os

import numpy as np

H = 2048          # hidden dim
F = 4096          # ffn dim per expert
E = 8             # experts
NCORES = 8
P = 128           # partitions
NH = H // P       # 16 h-tiles
FL = F // NCORES  # 512 ffn rows per expert per core
NFL = FL // P     # 4 f-tiles per expert per core

DT_MODE = os.environ.get("MOE_DT", "fp16")   # fp16 | bf16 | f32r | f32

# populated by kernel() for test harness introspection
LAST_STATS = {}

_BUILD_CACHE = {}


def _chunk_shape(count):
    """Capacity C >= count split into nch EQUAL even-width chunks <= 512
    (PSUM bank limit; even width keeps fp32r legal)."""
    c_min = max(2, count)
    n = max(1, math.ceil(c_min / 512))
    w0 = 2 * math.ceil(c_min / (2 * n))
    return n * w0, n, w0


def _build(schedule, dt_mode):
    """Build + compile the per-core Bass program.

    schedule: tuple of (C, nch, w0) per slot, processing order."""
    import concourse.bacc as bacc
    import concourse.mybir as mybir
    from concourse import tile

    AF = mybir.ActivationFunctionType
    f32 = mybir.dt.float32
    if dt_mode == "bf16":
        dmm = mybir.dt.bfloat16
    elif dt_mode == "fp16":
        dmm = mybir.dt.float16
    elif dt_mode == "f32":
        dmm = f32
    else:
        dmm = mybir.dt.float32r

    S = len(schedule)
    Ctot = sum(c for c, _, _ in schedule)
    offs = np.cumsum([0] + [c for c, _, _ in schedule]).tolist()

    nc = bacc.Bacc("TRN2", target_bir_lowering=False, debug=False)

    # Host-pre-tiled DRAM layouts (every DMA below is fully contiguous):
    #   w13t [S, NFL, P, 2, NH, P]  [s, fi, p, m, n, j] = w{1,3}_slice[fi*P+j, n*P+p]
    #   w2t  [S, P, NH, NFL, P]     [s, p, n, fi, j]    = w2_slice[fi*P+p, n*P+j]
    #   xt{s} [nch, P, NH, w0]      [ci, p, n, c]       = x_tok[ci*w0+c, n*P+p]
    #   wgtb [P, Ctot]              broadcast routing weights (slot-major cols)
    #   yt   [H, Ctot]              partial output
    w13_d = nc.dram_tensor("w13t", [S, NFL, P, 2, NH, P], dmm,
                           kind="ExternalInput").ap()
    w2_d = nc.dram_tensor("w2t", [S, P, NH, NFL, P], dmm,
                          kind="ExternalInput").ap()
    xt_ds = []
    for s, (C, nch, w0) in enumerate(schedule):
        xt_ds.append(nc.dram_tensor(f"xt{s}", [nch, P, NH, w0], dmm,
                                    kind="ExternalInput").ap())
    wg_d = nc.dram_tensor("wgtb", [P, Ctot], f32, kind="ExternalInput").ap()
    y_d = nc.dram_tensor("yt", [H, Ctot], f32, kind="ExternalOutput").ap()

    with tile.TileContext(nc) as tc:
        with (
            tc.tile_pool(name="inter", bufs=1) as inter_pool,
            tc.tile_pool(name="psum", bufs=2, space="PSUM") as psum_pool,
            tc.tile_pool(name="misc", bufs=1) as misc_pool,
            tc.tile_pool(name="xtp", bufs=4) as xt_pool,
            tc.tile_pool(name="wcol", bufs=3) as wcol_pool,
            tc.tile_pool(name="p1tmp", bufs=2) as p1tmp,
            tc.tile_pool(name="w2col", bufs=2) as w2_pool,
            tc.tile_pool(name="p2tmp", bufs=4) as p2tmp,
        ):
            # PE warmup: zero-matmuls with no DMA dependencies run
            # immediately, lifting the HAM clock gate (1.2 -> 2.4 GHz)
            # while the first real loads are still in flight.
            wsrc = misc_pool.tile([P, P], dmm, tag="wsrc")
            nc.vector.memset(wsrc[:], 0.0)
            wps = psum_pool.tile([P, 64], f32, tag="ps3", bufs=4,
                                 name="warm_ps")
            for i in range(80):
                nc.tensor.matmul(wps[:], wsrc[:], wsrc[:, :64],
                                 start=(i == 0), stop=(i == 79))

            # Startup critical path: interleave slot-0 chunk-a and the
            # first weight column on the SP queue in h-halves so the
            # first matmuls start as soon as the first halves land.
            C0, nch0, w00 = schedule[0]
            xt_tiles0 = [xt_pool.tile([P, NH, w00], dmm, tag="xt",
                                      name=f"xt0_{ci}")
                         for ci in range(nch0)]
            wc0 = wcol_pool.tile([P, 2, NH, P], dmm, tag="wc", name="wc0")
            h2 = NH // 2
            nc.sync.dma_start(xt_tiles0[0][:, :h2, :], xt_ds[0][0][:, :h2, :])
            nc.sync.dma_start(wc0[:, :, :h2, :], w13_d[0][0][:, :, :h2, :])
            nc.sync.dma_start(xt_tiles0[0][:, h2:, :], xt_ds[0][0][:, h2:, :])
            nc.sync.dma_start(wc0[:, :, h2:, :], w13_d[0][0][:, :, h2:, :])
            for ci in range(1, nch0):
                nc.sync.dma_start(xt_tiles0[ci][:], xt_ds[0][ci])

            wgtb = misc_pool.tile([P, Ctot], f32, tag="wgtb")
            wgtb_dma = nc.gpsimd.dma_start(wgtb[:], wg_d[:])
            wgtb_delayed = False

            for s, (C, nch, w0) in enumerate(schedule):
                par = s % 2
                off = offs[s]
                chunks = [(ci * w0, w0) for ci in range(nch)]

                # token tiles for this slot (slot 0 preloaded above)
                if s == 0:
                    xt_tiles = xt_tiles0
                else:
                    xt_tiles = []
                    for ci in range(nch):
                        xtc = xt_pool.tile([P, NH, w0], dmm, tag="xt",
                                           name=f"xt{s}_{ci}")
                        xt_tiles.append(xtc)
                        nc.scalar.dma_start(xtc[:], xt_ds[s][ci])

                # ---- phase 1: interT = silu(w1s @ xT) * (w3s @ xT) ----
                inter_tiles = []
                for fi in range(NFL):
                    if s == 0 and fi == 0:
                        wc = wc0
                    else:
                        wc = wcol_pool.tile([P, 2, NH, P], dmm, tag="wc")
                        wc_dma = nc.sync.dma_start(wc[:], w13_d[s][fi])
                        if not wgtb_delayed and s == 0 and fi == 2:
                            # wgtb is needed only in phase 2; keep it off
                            # the HBM-saturated startup window.
                            tile.add_dep_helper(
                                wgtb_dma.ins, wc_dma.ins,
                                reason="delay wgtb load past startup")
                            wgtb_delayed = True
                    it = inter_pool.tile([P, C], dmm, tag=f"it{par}_{fi}",
                                         name=f"inter{s}_{fi}")
                    inter_tiles.append(it)
                    # chunks interleaved per h-tile: consecutive matmuls
                    # share the stationary operand
                    ps1 = [psum_pool.tile([P, cw], f32, tag="ps1", bufs=4,
                                          name=f"ps1_{s}_{fi}_{ci}")
                           for ci, (c0, cw) in enumerate(chunks)]
                    ps3 = [psum_pool.tile([P, cw], f32, tag="ps3", bufs=4,
                                          name=f"ps3_{s}_{fi}_{ci}")
                           for ci, (c0, cw) in enumerate(chunks)]
                    for hi in range(NH):
                        for ci in range(nch):
                            nc.tensor.matmul(
                                ps1[ci][:], wc[:, 0, hi, :],
                                xt_tiles[ci][:, hi, :],
                                start=(hi == 0), stop=(hi == NH - 1))
                        for ci in range(nch):
                            nc.tensor.matmul(
                                ps3[ci][:], wc[:, 1, hi, :],
                                xt_tiles[ci][:, hi, :],
                                start=(hi == 0), stop=(hi == NH - 1))
                    for ci, (c0, cw) in enumerate(chunks):
                        # silu(a) = a * sigmoid(a)
                        sig = p1tmp.tile([P, cw], f32, tag="sig")
                        nc.scalar.activation(sig[:], ps1[ci][:], AF.Sigmoid)
                        sil = p1tmp.tile([P, cw], f32, tag="sil")
                        nc.vector.tensor_mul(sil[:], ps1[ci][:], sig[:])
                        nc.vector.tensor_mul(it[:, c0:c0 + cw], sil[:],
                                             ps3[ci][:])

                # w2 slice for this slot: queued on the SP ring behind the
                # slot's w13 columns, ahead of the next slot's -- lands
                # during phase 1, cannot crowd out startup.
                w2c = w2_pool.tile([P, NH, NFL, P], dmm, tag="w2c",
                                   name=f"w2c{s}")
                nc.sync.dma_start(w2c[:], w2_d[s])

                # ---- phase 2: yT[ht, :] = (w2s.T @ interT) * wgt ----
                for ht in range(NH):
                    ob = p2tmp.tile([P, C], f32, tag="ob")
                    po = [psum_pool.tile([P, cw], f32, tag="ps1", bufs=4,
                                         name=f"po_{s}_{ht}_{ci}")
                          for ci, (c0, cw) in enumerate(chunks)]
                    for fi in range(NFL):
                        for ci, (c0, cw) in enumerate(chunks):
                            nc.tensor.matmul(
                                po[ci][:], w2c[:, ht, fi, :],
                                inter_tiles[fi][:, c0:c0 + cw],
                                start=(fi == 0), stop=(fi == NFL - 1))
                    last = (s == S - 1) and (ht == NH - 1)
                    for ci, (c0, cw) in enumerate(chunks):
                        nc.vector.tensor_mul(ob[:, c0:c0 + cw], po[ci][:],
                                             wgtb[:, off + c0:off + c0 + cw])
                        if last:
                            # final h-tile: store per chunk so the last
                            # store overlaps the last matmul group
                            nc.gpsimd.dma_start(
                                y_d[ht * P:(ht + 1) * P,
                                    off + c0:off + c0 + cw],
                                ob[:, c0:c0 + cw])
                    if not last:
                        nc.gpsimd.dma_start(
                            y_d[ht * P:(ht + 1) * P, off:off + C], ob[:])

    nc.compile()
    return nc


def _get_nc(schedule, dt_mode):
    key = (tuple(schedule), dt_mode)
    if key not in _BUILD_CACHE:
        _BUILD_CACHE[key] = _build(key[0], dt_mode)
    return _BUILD_CACHE[key]


def _route(x, gate_w, top_k):
    """Host routing, matching the reference exactly:
    softmax(x @ gate_w.T) -> top-k (ties -> lower index) -> renormalize."""
    logits = x.astype(np.float64) @ gate_w.astype(np.float64).T
    m = logits.max(axis=-1, keepdims=True)
    p = np.exp(logits - m)
    p /= p.sum(axis=-1, keepdims=True)
    idx = np.argsort(-p, axis=-1, kind="stable")[:, :top_k]          # [T, k]
    vals = np.take_along_axis(p, idx, axis=-1)
    vals = vals / vals.sum(axis=-1, keepdims=True)
    return idx, vals.astype(np.float32)


def _fake_device(in_maps, schedule):
    """Numpy stand-in for the device: consumes the exact tiled in_maps
    (validates host-side layouts end-to-end). Dev aid, off by default."""
    class R:
        exec_time_ns = None
        mean_exec_time_ns = None
        instructions_and_trace = None
        results = []
    res = R()
    Ctot = sum(c for c, _, _ in schedule)
    offs = np.cumsum([0] + [c for c, _, _ in schedule]).tolist()
    for m in in_maps:
        yt = np.zeros((H, Ctot), np.float32)
        for s, (C, nch, w0) in enumerate(schedule):
            xs = m[f"xt{s}"].transpose(0, 3, 2, 1).reshape(C, H)
            xs = xs.astype(np.float32)
            w1e = m["w13t"][s][:, :, 0].transpose(0, 3, 2, 1).reshape(FL, H)
            w3e = m["w13t"][s][:, :, 1].transpose(0, 3, 2, 1).reshape(FL, H)
            w2e = m["w2t"][s].transpose(2, 0, 1, 3).reshape(FL, H)
            h1 = xs @ w1e.astype(np.float32).T
            h3 = xs @ w3e.astype(np.float32).T
            inter = (h1 / (1 + np.exp(-h1))) * h3
            y = inter @ w2e.astype(np.float32)                       # [C, H]
            wg = m["wgtb"][0, offs[s]:offs[s] + C]
            yt[:, offs[s]:offs[s] + C] = (y * wg[:, None]).T
        res.results.append({"yt": yt})
    return res


def kernel(x, gate_w, w1, w2, w3, top_k):
    from concourse.bass_utils import run_bass_kernel_spmd

    x = np.ascontiguousarray(np.asarray(x, dtype=np.float32))
    gate_w = np.asarray(gate_w, dtype=np.float32)
    w1 = np.asarray(w1, dtype=np.float32)
    w2 = np.asarray(w2, dtype=np.float32)
    w3 = np.asarray(w3, dtype=np.float32)
    k = int(np.asarray(top_k))
    t, h = x.shape
    e = gate_w.shape[0]
    f = w1.shape[0] // e
    assert (h, f, e) == (H, F, E), (h, f, e)

    dt_mode = DT_MODE
    import ml_dtypes
    np_mm = {"bf16": ml_dtypes.bfloat16, "fp16": np.float16}.get(
        dt_mode, np.float32)

    idx, vals = _route(x, gate_w, k)                                  # [T, k]

    # token lists per expert
    tok_lists = []
    wgt_lists = []
    for ei in range(E):
        tok_i, slot_i = np.nonzero(idx == ei)
        tok_lists.append(tok_i.astype(np.int64))
        wgt_lists.append(vals[tok_i, slot_i].astype(np.float32))
    counts = np.array([len(ti) for ti in tok_lists])
    order = np.argsort(-counts, kind="stable")      # largest slot first
    schedule = tuple(_chunk_shape(counts[e_]) for e_ in order)
    Ctot = sum(c for c, _, _ in schedule)
    offs = np.cumsum([0] + [c for c, _, _ in schedule]).tolist()

    xmm = x.astype(np_mm)

    # ---- replicated inputs: token tiles + routing weights ----
    common = {}
    wgt_full = np.zeros(Ctot, dtype=np.float32)
    for s, e_ in enumerate(order):
        C, nch, w0 = schedule[s]
        n = counts[e_]
        xs = np.zeros((C, H), dtype=np_mm)
        xs[:n] = xmm[tok_lists[e_]]
        common[f"xt{s}"] = np.ascontiguousarray(
            xs.reshape(nch, w0, NH, P).transpose(0, 3, 2, 1))
        wgt_full[offs[s]:offs[s] + n] = wgt_lists[e_]
    common["wgtb"] = np.ascontiguousarray(
        np.broadcast_to(wgt_full, (P, Ctot)).astype(np.float32))

    # ---- per-core weight slices ----
    in_maps = []
    for ci in range(NCORES):
        w13t = np.empty((len(order), NFL, P, 2, NH, P), dtype=np_mm)
        w2t = np.empty((len(order), P, NH, NFL, P), dtype=np_mm)
        for s, e_ in enumerate(order):
            r0 = e_ * F + ci * FL
            b1 = w1[r0:r0 + FL].astype(np_mm)
            b3 = w3[r0:r0 + FL].astype(np_mm)
            b2 = w2[r0:r0 + FL].astype(np_mm)
            # [s, fi, p, m, n, j] = w[fi*P+j, n*P+p]
            w13t[s] = np.stack(
                [b1.reshape(NFL, P, NH, P).transpose(0, 3, 2, 1),
                 b3.reshape(NFL, P, NH, P).transpose(0, 3, 2, 1)], axis=2)
            # [s, p, n, fi, j] = w2[fi*P+p, n*P+j]
            w2t[s] = b2.reshape(NFL, P, NH, P).transpose(1, 2, 0, 3)
        m = {"w13t": w13t, "w2t": w2t}
        m.update(common)
        in_maps.append(m)

    if os.environ.get("MOE_FAKE"):
        res = _fake_device(in_maps, schedule)
    else:
        nc = _get_nc(schedule, dt_mode)
        trace = bool(int(os.environ.get("MOE_TRACE", "0")))
        res = run_bass_kernel_spmd(nc, in_maps, core_ids=list(range(NCORES)),
                                   trace=trace)
    LAST_STATS.clear()
    iat = getattr(res, "instructions_and_trace", None)
    LAST_STATS.update({
        "schedule": schedule,
        "dt_mode": dt_mode,
        "exec_time_ns": res.exec_time_ns,
        "mean_exec_time_ns": res.mean_exec_time_ns,
        "counts": counts.tolist(),
        "trace": iat[1] if iat else None,
    })

    y = res.results[0]["yt"].astype(np.float32)
    for ci in range(1, NCORES):
        y += res.results[ci]["yt"]

    out = np.zeros((t, h), dtype=np.float32)
    for s, e_ in enumerate(order):
        n = counts[e_]
        out[tok_lists[e_]] += y[:, offs[s]:offs[s] + n].T.astype(np.float32)
    return out


# revision 8
# speedup vs baseline: 1.9053x; 1.9053x over previous
"""Block-sparse MoE (SwiGLU, top-k of 8 experts) on 8 Trainium2 NeuronCores.

Sharding: FFN-dim (tensor-parallel within every expert).
  - Routing (gate matmul + softmax + top-k, ~0.07% of total FLOPs) runs on
    the host; tokens are grouped per expert (exact counts, tiny padding).
  - Every core processes ALL expert token batches, but only a 512-row slice
    of each expert's w1/w3 (and the matching w2 rows). Work is therefore
    identical across cores regardless of the routing distribution -- the
    per-expert load imbalance of expert-parallel dispatch disappears
    (~9% of tensor time for this routing).
  - Core partial outputs are summed on the host, then scatter-added into
    the full [T, H] output.

Device kernel per core (slots s=0..7 are experts sorted by descending
token count C_s; FL=512 ffn rows per expert per core):
  phase 1: interT[f, c] = silu(w1s @ xT) * (w3s @ xT)   (PSUM-accum over H)
  phase 2: yT[h, c]     = (w2s.T @ interT) * wgt[c]     (PSUM-accum over FL)
Matmuls default to fp16 (measured end-to-end rel err ~5e-4). MOE_DT can
select bf16 / f32 / f32r. All DRAM->SBUF transfers are host-pre-tiled so
every DMA is contiguous.
"""

import math
import os

import numpy as np

H = 2048          # hidden dim
F = 4096          # ffn dim per expert
E = 8             # experts
NCORES = 8
P = 128           # partitions
NH = H // P       # 16 h-tiles
FL = F // NCORES  # 512 ffn rows per expert per core
NFL = FL // P     # 4 f-tiles per expert per core

DT_MODE = os.environ.get("MOE_DT", "fp16")   # fp16 | bf16 | f32r | f32

# populated by kernel() for test harness introspection
LAST_STATS = {}

_BUILD_CACHE = {}


def _chunk_shape(count):
    """Capacity C >= count split into nch EQUAL even-width chunks <= 512
    (PSUM bank limit; even width keeps fp32r legal)."""
    c_min = max(2, count)
    n = max(1, math.ceil(c_min / 512))
    w0 = 2 * math.ceil(c_min / (2 * n))
    return n * w0, n, w0


def _build(schedule, dt_mode):
    """Build + compile the per-core Bass program.

    schedule: tuple of (C, nch, w0) per slot, processing order."""
    import concourse.bacc as bacc
    import concourse.mybir as mybir
    from concourse import tile

    AF = mybir.ActivationFunctionType
    f32 = mybir.dt.float32
    if dt_mode == "bf16":
        dmm = mybir.dt.bfloat16
    elif dt_mode == "fp16":
        dmm = mybir.dt.float16
    elif dt_mode == "f32":
        dmm = f32
    else:
        dmm = mybir.dt.float32r

    S = len(schedule)
    Ctot = sum(c for c, _, _ in schedule)
    offs = np.cumsum([0] + [c for c, _, _ in schedule]).tolist()

    nc = bacc.Bacc("TRN2", target_bir_lowering=False, debug=False)

    # Host-pre-tiled DRAM layouts (every DMA below is fully contiguous):
    #   w13t [S, NFL, P, 2, NH, P]  [s, fi, p, m, n, j] = w{1,3}_slice[fi*P+j, n*P+p]
    #   w2t  [S, P, NH, NFL, P]     [s, p, n, fi, j]    = w2_slice[fi*P+p, n*P+j]
    #   xt{s} [nch, P, NH, w0]      [ci, p, n, c]       = x_tok[ci*w0+c, n*P+p]
    #   wgtb [P, Ctot]              broadcast routing weights (slot-major cols)
    #   yt   [H, Ctot]              partial output
    w13_d = nc.dram_tensor("w13t", [S, NFL, P, 2, NH, P], dmm,
                           kind="ExternalInput").ap()
    w2_d = nc.dram_tensor("w2t", [S, P, NH, NFL, P], dmm,
                          kind="ExternalInput").ap()
    xt_ds = []
    for s, (C, nch, w0) in enumerate(schedule):
        xt_ds.append(nc.dram_tensor(f"xt{s}", [nch, P, NH, w0], dmm,
                                    kind="ExternalInput").ap())
    wg_d = nc.dram_tensor("wgtb", [P, Ctot], f32, kind="ExternalInput").ap()
    # per-slot outputs, [p, n, c] = y[n*P+p, c]: long (NH*C) partition
    # lines keep the store path descriptor-cheap
    ydt = dmm if dt_mode in ("fp16", "bf16") else f32
    y_ds = [nc.dram_tensor(f"yt{s}", [P, NH, C], ydt,
                           kind="ExternalOutput").ap()
            for s, (C, nch, w0) in enumerate(schedule)]

    with tile.TileContext(nc) as tc:
        with (
            tc.tile_pool(name="inter", bufs=1) as inter_pool,
            tc.tile_pool(name="psum", bufs=2, space="PSUM") as psum_pool,
            tc.tile_pool(name="misc", bufs=1) as misc_pool,
            tc.tile_pool(name="xtp", bufs=4) as xt_pool,
            tc.tile_pool(name="wcol", bufs=3) as wcol_pool,
            tc.tile_pool(name="p1tmp", bufs=2) as p1tmp,
            tc.tile_pool(name="w2col", bufs=2) as w2_pool,
            tc.tile_pool(name="p2tmp", bufs=4) as p2tmp,
        ):
            # PE warmup: zero-matmuls with no DMA dependencies run
            # immediately, lifting the HAM clock gate (1.2 -> 2.4 GHz)
            # while the first real loads are still in flight.
            wsrc = misc_pool.tile([P, P], dmm, tag="wsrc")
            nc.vector.memset(wsrc[:], 0.0)
            wps = psum_pool.tile([P, 64], f32, tag="ps3", bufs=4,
                                 name="warm_ps")
            for i in range(80):
                nc.tensor.matmul(wps[:], wsrc[:], wsrc[:, :64],
                                 start=(i == 0), stop=(i == 79))

            # Startup critical path: interleave slot-0 chunk-a and the
            # first weight column on the SP queue in h-halves so the
            # first matmuls start as soon as the first halves land.
            C0, nch0, w00 = schedule[0]
            xt_tiles0 = [xt_pool.tile([P, NH, w00], dmm, tag="xt",
                                      name=f"xt0_{ci}")
                         for ci in range(nch0)]
            wc0 = wcol_pool.tile([P, 2, NH, P], dmm, tag="wc", name="wc0")
            h2 = NH // 2
            nc.sync.dma_start(xt_tiles0[0][:, :h2, :], xt_ds[0][0][:, :h2, :])
            nc.sync.dma_start(wc0[:, :, :h2, :], w13_d[0][0][:, :, :h2, :])
            nc.sync.dma_start(xt_tiles0[0][:, h2:, :], xt_ds[0][0][:, h2:, :])
            nc.sync.dma_start(wc0[:, :, h2:, :], w13_d[0][0][:, :, h2:, :])
            for ci in range(1, nch0):
                nc.sync.dma_start(xt_tiles0[ci][:], xt_ds[0][ci])

            wgtb = misc_pool.tile([P, Ctot], f32, tag="wgtb")
            wgtb_dma = nc.gpsimd.dma_start(wgtb[:], wg_d[:])
            wgtb_delayed = False

            for s, (C, nch, w0) in enumerate(schedule):
                par = s % 2
                off = offs[s]
                chunks = [(ci * w0, w0) for ci in range(nch)]

                # token tiles for this slot (slot 0 preloaded above)
                if s == 0:
                    xt_tiles = xt_tiles0
                else:
                    xt_tiles = []
                    for ci in range(nch):
                        xtc = xt_pool.tile([P, NH, w0], dmm, tag="xt",
                                           name=f"xt{s}_{ci}")
                        xt_tiles.append(xtc)
                        nc.scalar.dma_start(xtc[:], xt_ds[s][ci])

                # ---- phase 1: interT = silu(w1s @ xT) * (w3s @ xT) ----
                inter_tiles = []
                for fi in range(NFL):
                    if s == 0 and fi == 0:
                        wc = wc0
                    else:
                        wc = wcol_pool.tile([P, 2, NH, P], dmm, tag="wc")
                        wc_dma = nc.sync.dma_start(wc[:], w13_d[s][fi])
                        if not wgtb_delayed and s == 0 and fi == 2:
                            # wgtb is needed only in phase 2; keep it off
                            # the HBM-saturated startup window.
                            tile.add_dep_helper(
                                wgtb_dma.ins, wc_dma.ins,
                                reason="delay wgtb load past startup")
                            wgtb_delayed = True
                    it = inter_pool.tile([P, C], dmm, tag=f"it{par}_{fi}",
                                         name=f"inter{s}_{fi}")
                    inter_tiles.append(it)
                    # chunks interleaved per h-tile: consecutive matmuls
                    # share the stationary operand
                    ps1 = [psum_pool.tile([P, cw], f32, tag="ps1", bufs=4,
                                          name=f"ps1_{s}_{fi}_{ci}")
                           for ci, (c0, cw) in enumerate(chunks)]
                    ps3 = [psum_pool.tile([P, cw], f32, tag="ps3", bufs=4,
                                          name=f"ps3_{s}_{fi}_{ci}")
                           for ci, (c0, cw) in enumerate(chunks)]
                    for hi in range(NH):
                        for ci in range(nch):
                            nc.tensor.matmul(
                                ps1[ci][:], wc[:, 0, hi, :],
                                xt_tiles[ci][:, hi, :],
                                start=(hi == 0), stop=(hi == NH - 1))
                        for ci in range(nch):
                            nc.tensor.matmul(
                                ps3[ci][:], wc[:, 1, hi, :],
                                xt_tiles[ci][:, hi, :],
                                start=(hi == 0), stop=(hi == NH - 1))
                    for ci, (c0, cw) in enumerate(chunks):
                        # silu(a) = a * sigmoid(a)
                        sig = p1tmp.tile([P, cw], f32, tag="sig")
                        nc.scalar.activation(sig[:], ps1[ci][:], AF.Sigmoid)
                        sil = p1tmp.tile([P, cw], f32, tag="sil")
                        nc.vector.tensor_mul(sil[:], ps1[ci][:], sig[:])
                        nc.vector.tensor_mul(it[:, c0:c0 + cw], sil[:],
                                             ps3[ci][:])

                # w2 slice for this slot: queued on the SP ring behind the
                # slot's w13 columns, ahead of the next slot's -- lands
                # during phase 1, cannot crowd out startup.
                w2c = w2_pool.tile([P, NH, NFL, P], dmm, tag="w2c",
                                   name=f"w2c{s}")
                nc.sync.dma_start(w2c[:], w2_d[s])

                # ---- phase 2: yT[ht, :] = (w2s.T @ interT) * wgt ----
                # the whole slot's output accumulates in SBUF and goes out
                # as two big stores (h-halves) -- long partition lines keep
                # the store path descriptor-cheap
                ob = p2tmp.tile([P, NH, C], ydt, tag="ob", bufs=2,
                                name=f"ob{s}")
                hh = NH // 2
                for ht in range(NH):
                    po = [psum_pool.tile([P, cw], f32, tag="ps1", bufs=4,
                                         name=f"po_{s}_{ht}_{ci}")
                          for ci, (c0, cw) in enumerate(chunks)]
                    for fi in range(NFL):
                        for ci, (c0, cw) in enumerate(chunks):
                            nc.tensor.matmul(
                                po[ci][:], w2c[:, ht, fi, :],
                                inter_tiles[fi][:, c0:c0 + cw],
                                start=(fi == 0), stop=(fi == NFL - 1))
                    for ci, (c0, cw) in enumerate(chunks):
                        nc.vector.tensor_mul(ob[:, ht, c0:c0 + cw], po[ci][:],
                                             wgtb[:, off + c0:off + c0 + cw])
                    if ht == hh - 1:
                        nc.gpsimd.dma_start(y_ds[s][:, :hh, :],
                                            ob[:, :hh, :])
                    elif ht == NH - 1:
                        nc.gpsimd.dma_start(y_ds[s][:, hh:, :],
                                            ob[:, hh:, :])

    nc.compile()
    return nc


def _get_nc(schedule, dt_mode):
    key = (tuple(schedule), dt_mode)
    if key not in _BUILD_CACHE:
        _BUILD_CACHE[key] = _build(key[0], dt_mode)
    return _BUILD_CACHE[key]


def _route(x, gate_w, top_k):
    """Host routing, matching the reference exactly:
    softmax(x @ gate_w.T) -> top-k (ties -> lower index) -> renormalize."""
    logits = x.astype(np.float64) @ gate_w.astype(np.float64).T
    m = logits.max(axis=-1, keepdims=True)
    p = np.exp(logits - m)
    p /= p.sum(axis=-1, keepdims=True)
    idx = np.argsort(-p, axis=-1, kind="stable")[:, :top_k]          # [T, k]
    vals = np.take_along_axis(p, idx, axis=-1)
    vals = vals / vals.sum(axis=-1, keepdims=True)
    return idx, vals.astype(np.float32)


def _fake_device(in_maps, schedule):
    """Numpy stand-in for the device: consumes the exact tiled in_maps
    (validates host-side layouts end-to-end). Dev aid, off by default."""
    class R:
        exec_time_ns = None
        mean_exec_time_ns = None
        instructions_and_trace = None
        results = []
    res = R()
    offs = np.cumsum([0] + [c for c, _, _ in schedule]).tolist()
    for m in in_maps:
        outm = {}
        for s, (C, nch, w0) in enumerate(schedule):
            xs = m[f"xt{s}"].transpose(0, 3, 2, 1).reshape(C, H)
            xs = xs.astype(np.float32)
            w1e = m["w13t"][s][:, :, 0].transpose(0, 3, 2, 1).reshape(FL, H)
            w3e = m["w13t"][s][:, :, 1].transpose(0, 3, 2, 1).reshape(FL, H)
            w2e = m["w2t"][s].transpose(2, 0, 1, 3).reshape(FL, H)
            h1 = xs @ w1e.astype(np.float32).T
            h3 = xs @ w3e.astype(np.float32).T
            inter = (h1 / (1 + np.exp(-h1))) * h3
            y = inter @ w2e.astype(np.float32)                       # [C, H]
            wg = m["wgtb"][0, offs[s]:offs[s] + C]
            yt = (y * wg[:, None]).T                                 # [H, C]
            outm[f"yt{s}"] = np.ascontiguousarray(
                yt.reshape(NH, P, C).transpose(1, 0, 2))
        res.results.append(outm)
    return res


def kernel(x, gate_w, w1, w2, w3, top_k):
    from concourse.bass_utils import run_bass_kernel_spmd

    x = np.ascontiguousarray(np.asarray(x, dtype=np.float32))
    gate_w = np.asarray(gate_w, dtype=np.float32)
    w1 = np.asarray(w1, dtype=np.float32)
    w2 = np.asarray(w2, dtype=np.float32)
    w3 = np.asarray(w3, dtype=np.float32)
    k = int(np.asarray(top_k))
    t, h = x.shape
    e = gate_w.shape[0]
    f = w1.shape[0] // e
    assert (h, f, e) == (H, F, E), (h, f, e)

    dt_mode = DT_MODE
    import ml_dtypes
    np_mm = {"bf16": ml_dtypes.bfloat16, "fp16": np.float16}.get(
        dt_mode, np.float32)

    idx, vals = _route(x, gate_w, k)                                  # [T, k]

    # token lists per expert
    tok_lists = []
    wgt_lists = []
    for ei in range(E):
        tok_i, slot_i = np.nonzero(idx == ei)
        tok_lists.append(tok_i.astype(np.int64))
        wgt_lists.append(vals[tok_i, slot_i].astype(np.float32))
    counts = np.array([len(ti) for ti in tok_lists])
    order = np.argsort(-counts, kind="stable")      # largest slot first
    schedule = tuple(_chunk_shape(counts[e_]) for e_ in order)
    Ctot = sum(c for c, _, _ in schedule)
    offs = np.cumsum([0] + [c for c, _, _ in schedule]).tolist()

    xmm = x.astype(np_mm)

    # ---- replicated inputs: token tiles + routing weights ----
    common = {}
    wgt_full = np.zeros(Ctot, dtype=np.float32)
    for s, e_ in enumerate(order):
        C, nch, w0 = schedule[s]
        n = counts[e_]
        xs = np.zeros((C, H), dtype=np_mm)
        xs[:n] = xmm[tok_lists[e_]]
        common[f"xt{s}"] = np.ascontiguousarray(
            xs.reshape(nch, w0, NH, P).transpose(0, 3, 2, 1))
        wgt_full[offs[s]:offs[s] + n] = wgt_lists[e_]
    common["wgtb"] = np.ascontiguousarray(
        np.broadcast_to(wgt_full, (P, Ctot)).astype(np.float32))

    # ---- per-core weight slices ----
    in_maps = []
    for ci in range(NCORES):
        w13t = np.empty((len(order), NFL, P, 2, NH, P), dtype=np_mm)
        w2t = np.empty((len(order), P, NH, NFL, P), dtype=np_mm)
        for s, e_ in enumerate(order):
            r0 = e_ * F + ci * FL
            b1 = w1[r0:r0 + FL].astype(np_mm)
            b3 = w3[r0:r0 + FL].astype(np_mm)
            b2 = w2[r0:r0 + FL].astype(np_mm)
            # [s, fi, p, m, n, j] = w[fi*P+j, n*P+p]
            w13t[s] = np.stack(
                [b1.reshape(NFL, P, NH, P).transpose(0, 3, 2, 1),
                 b3.reshape(NFL, P, NH, P).transpose(0, 3, 2, 1)], axis=2)
            # [s, p, n, fi, j] = w2[fi*P+p, n*P+j]
            w2t[s] = b2.reshape(NFL, P, NH, P).transpose(1, 2, 0, 3)
        m = {"w13t": w13t, "w2t": w2t}
        m.update(common)
        in_maps.append(m)

    if os.environ.get("MOE_FAKE"):
        res = _fake_device(in_maps, schedule)
    else:
        nc = _get_nc(schedule, dt_mode)
        trace = bool(int(os.environ.get("MOE_TRACE", "0")))
        res = run_bass_kernel_spmd(nc, in_maps, core_ids=list(range(NCORES)),
                                   trace=trace)
    LAST_STATS.clear()
    iat = getattr(res, "instructions_and_trace", None)
    LAST_STATS.update({
        "schedule": schedule,
        "dt_mode": dt_mode,
        "exec_time_ns": res.exec_time_ns,
        "mean_exec_time_ns": res.mean_exec_time_ns,
        "counts": counts.tolist(),
        "trace": iat[1] if iat else None,
    })

    out = np.zeros((t, h), dtype=np.float32)
    for s, e_ in enumerate(order):
        C = schedule[s][0]
        n = counts[e_]
        ys = res.results[0][f"yt{s}"].astype(np.float32)
        for ci in range(1, NCORES):
            ys += res.results[ci][f"yt{s}"]
        # ys [P, NH, C] -> y [H, C] with h = n*P + p
        yh = ys.transpose(1, 0, 2).reshape(H, C)
        out[tok_lists[e_]] += yh[:, :n].T
    return out
